# revision 2
# baseline (speedup 1.0000x reference)
"""GATv2 + Bessel edge-softmax kernel for TRN2, 8-core SPMD, dst-sharded. v2.

Key structure (vs v1 baseline):
  - Degree-balanced node->slot permutation equalizes edges per 128-node window.
  - Feature tables hold 512B rows: [feat (f,h)-transposed 128 | q=sum_f attn*feat
    (4) | pad].  512B rows cost the same DMA as 256B (sub-512B pays 2x latency),
    so the q columns ride free.
  - PReLU linearized: attn . prelu(c*s) = p*c*(Q) + q*|c|*(sum_f attn*|s|) with
    Q = qs[src]+qd[dst] gathered, c1=p*c / c2=q*|c| host-precomputed per edge.
  - (f,h)-transposed layout lets the per-edge scalars (ex) multiply at DVE 2x
    via mid-dim broadcast views; no ACT materializations of big broadcasts.
  - One-hot scatter matrix built on the Pool engine; abs on Activation engine.
  - 48KB SWDGE scratch ring -> whole-group gather calls (4 calls/window).
"""
import sys
sys.path.insert(0, "/opt/trn_rl_repo")
import heapq
import numpy as np
import ml_dtypes
import concourse.bass as bass
import concourse.tile as tile
from concourse import bacc, mybir
from concourse.bass import ts
from contextlib import ExitStack

F32 = mybir.dt.float32
BF = mybir.dt.bfloat16
I16 = mybir.dt.int16
BFNP = ml_dtypes.bfloat16

CUTOFF = 4.0
P_ENV = 7
H, F, HF, IN = 4, 32, 128, 128
ROW = 256          # table row stride/gather elem (in bf16 elems) = 512B
QC = HF            # column where q values start in a row
NCOL = HF + H      # used columns per row


class Cfg:
    def __init__(self, N, n_cores, NV, B_lo, B_hi, V_lo):
        self.N = N
        self.n_cores = n_cores
        self.NV = NV
        self.NW = NV // 128
        self.N_pad = NV * n_cores
        self.B_lo = B_lo
        self.B_hi = B_hi
        self.B = B_lo + B_hi
        self.V_lo = V_lo
        self.V_hi = self.N_pad - V_lo
        assert self.V_lo % 128 == 0 and self.V_lo <= 32768
        assert self.V_hi <= 32768
        self.scratch = getattr(Cfg, "SCRATCH", 16384)  # power-of-2 ring carveout
        # SWDGE ucode caps one gather call at 1024 descriptors (8 blocks)
        GB = min(8, self.scratch // 16 // 128 // getattr(Cfg, "RING_DIV", 1))
        plan = []                       # (which, blk0, nblk, idx_col0)
        col = 0
        for which, total in (("lo", self.B_lo), ("hi", self.B_hi), ("er", self.B)):
            n = (total + GB - 1) // GB
            base, rem = total // n, total % n
            b0 = 0
            for i in range(n):
                nb = base + (1 if i < rem else 0)
                plan.append((which, b0, nb, col))
                col += 8 * nb
                b0 += nb
        self.plan = plan
        self.S = col


def balance_slots(dst, deg_lo, deg_hi, N, n_cores, NV):
    """Assign nodes to (window, partition) slots, vector-LPT balancing the
    per-window lo-edge and hi-edge counts simultaneously (lazy max-norm heap)."""
    NW = NV // 128
    n_win = n_cores * NW
    deg = deg_lo + deg_hi
    avg_lo = max(1.0, deg_lo.sum() / n_win)
    avg_hi = max(1.0, deg_hi.sum() / n_win)
    order = np.argsort(-deg, kind="stable")
    lo_s = np.zeros(n_win)
    hi_s = np.zeros(n_win)
    counts = np.zeros(n_win, np.int32)
    key = lambda wi: max(lo_s[wi] / avg_lo, hi_s[wi] / avg_hi)
    heap = [(0.0, wi) for wi in range(n_win)]
    heapq.heapify(heap)
    slot_of = np.empty(N, np.int64)
    for n in order:
        while True:
            k, wi = heapq.heappop(heap)
            kk = key(wi)
            if heap and kk > heap[0][0] + 1e-12:
                heapq.heappush(heap, (kk, wi))
                continue
            break
        slot_of[n] = wi * 128 + counts[wi]
        counts[wi] += 1
        lo_s[wi] += deg_lo[n]
        hi_s[wi] += deg_hi[n]
        if counts[wi] < 128:
            heapq.heappush(heap, (key(wi), wi))
    return slot_of


def pick_cfg(src, dst, N, n_cores=8):
    NV = ((N + n_cores * 128 - 1) // (n_cores * 128)) * 128
    N_pad = NV * n_cores
    V_lo = min(32768, N_pad)
    assert N_pad - V_lo <= 32768
    srcv, dstv = np.asarray(src), np.asarray(dst)
    is_lo = srcv < V_lo
    deg_lo = np.bincount(dstv[is_lo], minlength=N).astype(np.float64)
    deg_hi = np.bincount(dstv[~is_lo], minlength=N).astype(np.float64)
    slot_of = balance_slots(dstv, deg_lo, deg_hi, N, n_cores, NV)
    dslot = slot_of[dstv]
    win = dslot // 128
    n_lo = np.bincount(win[is_lo], minlength=n_cores * (NV // 128))
    n_hi = np.bincount(win[~is_lo], minlength=n_cores * (NV // 128))
    B_lo = int((n_lo.max() + 127) // 128)
    B_hi = int(max(1, (n_hi.max() + 127) // 128))
    cfg = Cfg(N, n_cores, NV, B_lo, B_hi, V_lo)
    cfg.slot_of = slot_of
    return cfg


def _coeff(distance, frequencies):
    d = (distance.astype(np.float64) / CUTOFF)[:, None]
    d7 = d ** P_ENV
    A = -(P_ENV + 1) * (P_ENV + 2) / 2.0
    Bc = float(P_ENV * (P_ENV + 2))
    C = -P_ENV * (P_ENV + 1) / 2.0
    env = d + A * d7 + Bc * (d7 * d) + C * (d7 * d * d)
    return (env * np.sin(frequencies.astype(np.float64) * d)).astype(np.float64)


def wrap_idx(vals, nslots):
    """SWDGE idx layout for ONE gather call of `nslots` idxs:
    [16, nslots/16] wrap replicated over the 8 gpsimd groups -> [128, nslots/16]."""
    a = np.zeros(nslots, np.int32)
    a[: len(vals)] = vals
    w = a.reshape(nslots // 16, 16).T.astype(np.int16)
    return np.tile(w, (8, 1))


def _perm_cols(W, b, attn):
    """[IN,NCOL] projection matrix: cols f*H+h = W[h*F+f,:]; cols QC+h = attn-dot."""
    Wext = np.zeros((IN, ROW), np.float64)
    bext = np.zeros((ROW,), np.float64)
    W = W.astype(np.float64)
    b = b.astype(np.float64)
    at = attn.reshape(H, F).astype(np.float64)
    for h in range(H):
        for f in range(F):
            Wext[:, f * H + h] = W[h * F + f, :]
            bext[f * H + h] = b[h * F + f]
        Wext[:, QC + h] = (at[h, :, None] * W[h * F : (h + 1) * F, :]).sum(axis=0)
        bext[QC + h] = (at[h] * b[h * F : (h + 1) * F]).sum()
    return Wext, bext


def host_prep(x, distance, W_src, b_src, W_dst, b_dst, attn, prelu_alpha,
              frequencies, src, dst, cfg: Cfg):
    c = cfg
    src = np.asarray(src).astype(np.int64)
    dst = np.asarray(dst).astype(np.int64)
    slot_of = c.slot_of
    dslot = slot_of[dst]
    order = np.argsort(dslot, kind="stable")
    src_s, dslot_s, dis_s = src[order], dslot[order], np.asarray(distance)[order]

    cfg.has_bias = bool(np.any(np.asarray(b_src)) or np.any(np.asarray(b_dst)))
    coeff = _coeff(dis_s, np.asarray(frequencies))          # [E, H] float64
    alpha = np.asarray(prelu_alpha).astype(np.float64)
    p = (1.0 + alpha) / 2.0
    q = (1.0 - alpha) / 2.0
    c1_all = (p[None, :] * coeff).astype(np.float32)
    c2_all = (q[None, :] * np.abs(coeff)).astype(np.float32)

    # node -> slot inverse for x column placement
    node_of = np.full(c.N_pad, -1, np.int64)
    node_of[slot_of] = np.arange(c.N)

    xT = np.zeros((IN, c.N_pad), BFNP)
    xT[:, : c.N] = np.asarray(x).T.astype(BFNP)

    Wse, bse = _perm_cols(np.asarray(W_src), np.asarray(b_src), np.asarray(attn))
    Wde, bde = _perm_cols(np.asarray(W_dst), np.asarray(b_dst), np.asarray(attn))
    attn_t = np.zeros((1, HF), np.float32)
    at = np.asarray(attn).reshape(H, F)
    for h in range(H):
        for f in range(F):
            attn_t[0, f * H + h] = at[h, f]

    smalls = dict(
        w_src_e=Wse[:, :NCOL].astype(BFNP),
        w_dst_e=Wde[:, :NCOL].astype(BFNP),
        b_src_e=bse[None, :NCOL].astype(BFNP),
        b_dst_e=bde[None, :NCOL].astype(BFNP),
        attn_t_row=attn_t.astype(BFNP),
    )

    core_of = dslot_s // c.NV
    maps = []
    for k in range(c.n_cores):
        sel = core_of == k
        sk = src_s[sel]
        lk = dslot_s[sel] - k * c.NV          # local slot
        c1k, c2k = c1_all[sel], c2_all[sel]
        win = lk // 128
        part = lk % 128

        idx = np.zeros((c.NW, 128, c.S), np.int16)
        dstw = np.full((128, c.NW, c.B), -1.0, BFNP)
        c12 = np.zeros((128, c.NW, c.B, 2 * H), np.float32)

        for w in range(c.NW):
            wsel = win == w
            ws, wp = sk[wsel], part[wsel]
            w1, w2 = c1k[wsel], c2k[wsel]
            lo = ws < c.V_lo
            n_lo, n_hi = int(lo.sum()), int((~lo).sum())
            assert n_lo <= c.B_lo * 128 and n_hi <= c.B_hi * 128

            # edge slot j -> (p=j%128, blk=j//128); lo slots then hi slots
            ers = np.zeros(c.B * 128, np.int64)   # er gather idx (local dst slot)
            jl = np.arange(n_lo)
            dstw[jl % 128, w, jl // 128] = wp[lo]
            c12[jl % 128, w, jl // 128, 0:H] = w1[lo]
            c12[jl % 128, w, jl // 128, H:] = w2[lo]
            ers[jl] = w * 128 + wp[lo]
            jh = np.arange(n_hi)
            dstw[jh % 128, w, c.B_lo + jh // 128] = wp[~lo]
            c12[jh % 128, w, c.B_lo + jh // 128, 0:H] = w1[~lo]
            c12[jh % 128, w, c.B_lo + jh // 128, H:] = w2[~lo]
            ers[c.B_lo * 128 + jh] = w * 128 + wp[~lo]

            els_lo = np.zeros(c.B_lo * 128, np.int64)
            els_lo[:n_lo] = ws[lo]
            els_hi = np.zeros(c.B_hi * 128, np.int64)
            els_hi[:n_hi] = ws[~lo] - c.V_lo
            pools = {"lo": els_lo, "hi": els_hi, "er": ers}
            for which, b0, nb, col in c.plan:
                vals = pools[which][b0 * 128 : (b0 + nb) * 128]
                idx[w, :, col : col + 8 * nb] = wrap_idx(vals, nb * 128)

        m = dict(smalls)
        xT_own = np.zeros((IN, c.NV), BFNP)
        own_nodes = node_of[k * c.NV : (k + 1) * c.NV]
        valid = own_nodes >= 0
        xT_own[:, valid] = xT[:, own_nodes[valid]]
        m.update(xT=xT, xT_own=xT_own, idx=idx, dstw=dstw, c12=c12)
        maps.append(m)
    return maps


def build_kernel(c: Cfg):
    nc = bacc.Bacc("TRN2", target_bir_lowering=False, debug=False,
                   dynamic_dma_scratch_size=c.scratch)
    dp = nc.declare_dram_parameter
    xT = dp("xT", [IN, c.N_pad], BF, isOutput=False)
    xT_own = dp("xT_own", [IN, c.NV], BF, isOutput=False)
    w_src_e = dp("w_src_e", [IN, NCOL], BF, isOutput=False)
    w_dst_e = dp("w_dst_e", [IN, NCOL], BF, isOutput=False)
    b_src_e = dp("b_src_e", [1, NCOL], BF, isOutput=False)
    b_dst_e = dp("b_dst_e", [1, NCOL], BF, isOutput=False)
    attn_t_row = dp("attn_t_row", [1, HF], BF, isOutput=False)
    idx_d = dp("idx", [c.NW, 128, c.S], I16, isOutput=False)
    dstw = dp("dstw", [128, c.NW, c.B], BF, isOutput=False)
    c12d = dp("c12", [128, c.NW, c.B, 2 * H], F32, isOutput=False)
    out = dp("out", [c.NV, HF], F32, isOutput=True)

    if getattr(c, "host_tables", False):
        feat_lo = dp("feat_lo", [c.V_lo, ROW], BF, isOutput=False)
        feat_hi = dp("feat_hi", [c.V_hi, ROW], BF, isOutput=False)
        feat_dst = dp("feat_dst", [c.NV, ROW], BF, isOutput=False)
        c.skip_proj = True
    else:
        feat_lo = nc.dram_tensor("feat_lo", [c.V_lo, ROW], BF)
        feat_hi = nc.dram_tensor("feat_hi", [c.V_hi, ROW], BF)
        feat_dst = nc.dram_tensor("feat_dst", [c.NV, ROW], BF)

    debug = getattr(c, "debug", False)
    if debug:
        dbg_el2 = dp("dbg_el2", [128, c.B, ROW], BF, isOutput=True)
        dbg = {name: dp(f"dbg_{name}", shape, dt, isOutput=True)
               for name, shape, dt in [
                   ("el", [128, c.B, ROW], BF), ("er", [128, c.B, ROW], BF),
                   ("s", [128, c.B, HF], BF), ("sh2", [128, c.B, 32], BF),
                   ("bred", [128, c.B, H], F32), ("qa", [128, c.B, H], BF),
                   ("score", [128, c.B, H], F32), ("msgex", [128, c.B, NCOL], BF),
                   ("oh", [128, c.B, 128], BF), ("U", [128, NCOL], F32)]}

    mm = mybir.AluOpType
    AF = mybir.ActivationFunctionType

    def apv(base_ap, dims):
        """Rebuild an AP view on the same tensor/offset with custom free dims."""
        return bass.AP(tensor=base_ap.tensor, offset=base_ap.offset,
                       ap=[list(base_ap.ap[0])] + [list(d) for d in dims])

    with tile.TileContext(nc, pool_alloc_mode="queue") as tc, ExitStack() as ctx:
        con = ctx.enter_context(tc.tile_pool(name="con", bufs=1))
        attn_rep = con.tile([128, HF], BF)
        nc.sync.dma_start(
            out=attn_rep[:],
            in_=bass.AP(tensor=attn_t_row.ap().tensor, offset=attn_t_row.ap().offset,
                        ap=[[0, 128], [1, HF]]))
        iota_i = con.tile([128, 128], mybir.dt.int32)
        nc.gpsimd.iota(iota_i[:], pattern=[[1, 128]], base=0, channel_multiplier=0)
        iota_f = con.tile([128, 128], BF)
        nc.vector.tensor_copy(out=iota_f[:], in_=iota_i[:])
        ones_sb = con.tile([1, 128], BF)
        nc.vector.memset(ones_sb[:], 1.0)
        dstw_sb = con.tile([128, c.NW, c.B], BF)
        nc.sync.dma_start(out=dstw_sb[:], in_=dstw[:])

        # --- projections: feat tables with q columns ---
        skip_proj = getattr(c, "skip_proj", False)
        skip_edges = getattr(c, "skip_edges", False)
        with tc.tile_pool(name="proj", bufs=3) as pp, \
             tc.tile_pool(name="projp", bufs=2, space="PSUM") as ppp:
            w_src_sb = pp.tile([IN, NCOL], BF, tag="wsrc")
            nc.sync.dma_start(out=w_src_sb[:], in_=w_src_e[:])
            w_dst_sb = pp.tile([IN, NCOL], BF, tag="wdst")
            nc.sync.dma_start(out=w_dst_sb[:], in_=w_dst_e[:])
            b_src_sb = pp.tile([1, NCOL], BF, tag="bsrc")
            nc.sync.dma_start(out=b_src_sb[:], in_=b_src_e[:])
            b_dst_sb = pp.tile([1, NCOL], BF, tag="bdst")
            nc.sync.dma_start(out=b_dst_sb[:], in_=b_dst_e[:])

            last_write = {}       # table name -> last write DMA instruction

            def project(xt_ap, n_tiles, w_sb, b_sb, dests, has_bias):
                G = 4
                for g0 in range(0, n_tiles, G):
                    g = min(G, n_tiles - g0)
                    xt_t = pp.tile([128, G * 128], BF, tag="xt")
                    nc.sync.dma_start(out=xt_t[:, : g * 128],
                                      in_=xt_ap[:, g0 * 128 : (g0 + g) * 128])
                    # 256-col (1KB) stride per tile keeps every matmul output
                    # region 512B-aligned in PSUM
                    ps = ppp.tile([128, G, 256], F32)
                    for t in range(g):
                        nc.tensor.matmul(ps[:, t, :NCOL], lhsT=xt_t[:, ts(t, 128)],
                                         rhs=w_sb[:], start=True, stop=not has_bias)
                        if has_bias:
                            nc.tensor.matmul(ps[:, t, :NCOL], lhsT=ones_sb[:],
                                             rhs=b_sb[:], start=False, stop=True)
                    # full-ROW rows (pad cols carry garbage, never read) so the
                    # table write is a contiguous row-major DMA
                    ft = pp.tile([128, G, ROW], BF, tag="ft")
                    nc.scalar.copy(
                        out=ft[:, :g, :NCOL],
                        in_=ps[:, :g, :NCOL])
                    for dram, t0, nt in dests:
                        a = max(g0, t0)
                        b = min(g0 + g, t0 + nt)
                        if a < b:
                            rows = dram[(a - t0) * 128 : (b - t0) * 128, :]
                            last_write[dram.name] = nc.sync.dma_start(
                                out=rows.rearrange("(t p) f -> p t f", p=128),
                                in_=ft[:, a - g0 : b - g0, :])
            if not skip_proj:
                hb = getattr(c, "has_bias", True)
                project(xT_own.ap(), c.NV // 128, w_dst_sb, b_dst_sb,
                        [(feat_dst, 0, c.NV // 128)], hb)
                project(xT.ap(), c.N_pad // 128, w_src_sb, b_src_sb,
                        [(feat_lo, 0, c.V_lo // 128),
                         (feat_hi, c.V_lo // 128, c.V_hi // 128)], hb)


        # --- edge phase ---
        ep = ctx.enter_context(tc.tile_pool(name="edge", bufs=2))
        wp = ctx.enter_context(tc.tile_pool(name="work", bufs=1))
        mp = ctx.enter_context(tc.tile_pool(name="mpool", bufs=2))
        dp2 = ctx.enter_context(tc.tile_pool(name="dwpool", bufs=2))
        op_ = ctx.enter_context(tc.tile_pool(name="outp", bufs=2))
        up = ctx.enter_context(tc.tile_pool(name="upsum", bufs=2, space="PSUM"))

        B = c.B
        for w in range(c.NW if not skip_edges else 0):
            id_t = ep.tile([128, c.S], I16, tag="idx")
            nc.sync.dma_start(out=id_t[:], in_=idx_d[w])
            c12w = ep.tile([128, B, 2 * H], F32, tag="c12w")
            nc.sync.dma_start(out=c12w[:], in_=c12d[:, w])

            el = ep.tile([128, B, ROW], BF, tag="el")
            er = ep.tile([128, B, ROW], BF, tag="er")
            for which, b0, nb, col in c.plan:
                if which == "lo":
                    dst_sl, tab = el[:, b0 : b0 + nb, :], feat_lo
                elif which == "hi":
                    dst_sl, tab = el[:, c.B_lo + b0 : c.B_lo + b0 + nb, :], feat_hi
                else:
                    dst_sl, tab = er[:, b0 : b0 + nb, :], feat_dst
                g_inst = nc.gpsimd.dma_gather(dst_sl, tab[:],
                                              id_t[:, col : col + 8 * nb],
                                              nb * 128, nb * 128, ROW)
                # dma_gather's DRAM source is not dependency-tracked by the
                # tile framework: order it after that table's final write
                lw = last_write.get(tab.name)
                if lw is not None:
                    tile.add_dep_helper(
                        g_inst.ins if hasattr(g_inst, "ins") else g_inst,
                        lw.ins if hasattr(lw, "ins") else lw,
                        reason="gather after table write")

            # one-hot: oh[p, b, n] = (iota[n] == dstw[p, b]); dstw broadcast
            # materialized on ACT so the DVE is_equal stays in 2x mode
            dwbig = dp2.tile([128, B, 128], BF, tag="dwbig")
            dw = dstw_sb[:, w, :]                       # [128, B]
            nc.scalar.copy(out=dwbig[:],
                           in_=apv(dw, [list(dw.ap[1]), [0, 128]]))
            oh = mp.tile([128, B, 128], BF, tag="oh")
            nc.vector.tensor_tensor(
                out=oh[:],
                in0=apv(iota_f[:], [[0, B], [1, 128]]),
                in1=dwbig[:],
                op=mm.is_equal)

            # s = el + er (feat part), DVE 2x; then |s| in place on ACT;
            # then *= attn in place (2x, mid-bcast of replicated attn row)
            s_t = wp.tile([128, B, HF], BF, tag="s")
            nc.vector.tensor_add(s_t[:], el[:, :, :HF], er[:, :, :HF])
            nc.scalar.activation(s_t[:], s_t[:], AF.Abs)
            nc.vector.tensor_tensor(
                out=s_t[:], in0=s_t[:],
                in1=apv(attn_rep[:], [[0, B], [1, HF]]), op=mm.mult)
            u_t = s_t
            # pairwise halving (f-major layout: f and f+16 are 64 cols apart)
            sh1 = wp.tile([128, B, 64], BF, tag="sh1")
            nc.vector.tensor_add(sh1[:], u_t[:, :, :64], u_t[:, :, 64:])
            sh2 = wp.tile([128, B, 32], BF, tag="sh2")
            nc.vector.tensor_add(sh2[:], sh1[:, :, :32], sh1[:, :, 32:])
            # reduce over f (strided view [p, B, h, f8]) -> Bred [128, B, H] f32
            bred = wp.tile([128, B, H], F32, tag="bred")
            sh2b = sh2[:]
            nc.vector.tensor_reduce(
                out=bred[:],
                in_=bass.AP(tensor=sh2b.tensor, offset=sh2b.offset,
                            ap=[list(sh2b.ap[0]), [32, B], [1, H], [H, 8]]),
                axis=mybir.AxisListType.X, op=mm.add)

            # score = c1*(qs+qd) + c2*Bred   [128, B, H] f32
            qa = wp.tile([128, B, H], BF, tag="qa")
            nc.vector.tensor_add(qa[:], el[:, :, QC:NCOL], er[:, :, QC:NCOL])
            sc1 = wp.tile([128, B, H], F32, tag="sc1")
            nc.vector.tensor_tensor(out=sc1[:], in0=qa[:], in1=c12w[:, :, 0:H],
                                    op=mm.mult)
            score = wp.tile([128, B, H], F32, tag="score")
            # fold the final add into the c2 multiply via scalar_tensor_tensor:
            # score = (bred * 1.0 ... ) no scalar slot for c2 — keep 2-op form
            sc2 = wp.tile([128, B, H], F32, tag="sc2")
            nc.vector.tensor_tensor(out=sc2[:], in0=bred[:], in1=c12w[:, :, H:],
                                    op=mm.mult)
            nc.vector.tensor_add(score[:], sc1[:], sc2[:])

            # msgex: cols 0:HF = el*ex, cols HF:NCOL = ex
            msgex = mp.tile([128, B, NCOL], BF, tag="msgex")
            nc.scalar.activation(msgex[:, :, QC:NCOL], score[:], AF.Exp)
            exv = msgex[:, :, QC:NCOL]
            nc.vector.tensor_tensor(
                out=msgex[:, :, :HF], in0=el[:, :, :HF],
                in1=bass.AP(tensor=exv.tensor, offset=exv.offset,
                            ap=[list(exv.ap[0]), [NCOL, B], [0, F], [1, H]]),
                op=mm.mult)

            # scatter via one-hot matmuls
            U = up.tile([128, NCOL], F32, tag="U")
            for b in range(B):
                nc.tensor.matmul(U[:], lhsT=oh[:, b, :], rhs=msgex[:, b, :],
                                 start=(b == 0), stop=(b == B - 1))

            if debug and w == 0:
                for nm, t in [("el", el), ("er", er), ("s", u_t), ("sh2", sh2),
                              ("bred", bred), ("qa", qa), ("score", score),
                              ("msgex", msgex), ("oh", oh)]:
                    nc.sync.dma_start(out=dbg[nm][:], in_=t[:])
                uc = op_.tile([128, NCOL], F32, tag="uc")
                nc.scalar.copy(out=uc[:], in_=U[:])
                nc.sync.dma_start(out=dbg["U"][:], in_=uc[:])

            inv = op_.tile([128, H], F32, tag="inv")
            nc.vector.tensor_scalar_max(inv[:], U[:, QC:NCOL], 1e-30)
            nc.vector.reciprocal(inv[:], inv[:])
            # ot holds (h,f) layout: strided DVE out unscrambles the (f,h) cols
            ot = op_.tile([128, HF], F32, tag="ot")
            otb = ot[:]
            nc.vector.tensor_tensor(
                out=bass.AP(tensor=otb.tensor, offset=otb.offset,
                            ap=[list(otb.ap[0]), [1, F], [F, H]]),
                in0=U[:, :HF],
                in1=apv(inv[:], [[0, F], [1, H]]), op=mm.mult)
            nc.sync.dma_start(out=out[ts(w, 128)], in_=ot[:])

        if debug:
            # re-gather window 0's el at the very end: distinguishes a
            # write/gather race from permanently corrupt table rows
            id2 = ep.tile([128, c.S], I16, tag="idx")
            nc.sync.dma_start(out=id2[:], in_=idx_d[0])
            el2 = ep.tile([128, B, ROW], BF, tag="el")
            for which, b0, nb, col in c.plan:
                if which == "lo":
                    nc.gpsimd.dma_gather(el2[:, b0 : b0 + nb, :], feat_lo[:],
                                         id2[:, col : col + 8 * nb],
                                         nb * 128, nb * 128, ROW)
                elif which == "hi":
                    nc.gpsimd.dma_gather(el2[:, c.B_lo + b0 : c.B_lo + b0 + nb, :],
                                         feat_hi[:], id2[:, col : col + 8 * nb],
                                         nb * 128, nb * 128, ROW)
            nc.sync.dma_start(out=dbg_el2[:], in_=el2[:])

    nc.compile()
    return nc


def kernel(**inputs) -> np.ndarray:
    x = np.asarray(inputs["x"], np.float32)
    src = np.asarray(inputs["src"]).astype(np.int64)
    dst = np.asarray(inputs["dst"]).astype(np.int64)
    cfg = pick_cfg(src, dst, x.shape[0], 8)
    maps = host_prep(
        x, np.asarray(inputs["distance"], np.float32),
        np.asarray(inputs["W_src"], np.float32), np.asarray(inputs["b_src"], np.float32),
        np.asarray(inputs["W_dst"], np.float32), np.asarray(inputs["b_dst"], np.float32),
        np.asarray(inputs["attn"], np.float32), np.asarray(inputs["prelu_alpha"], np.float32),
        np.asarray(inputs["frequencies"], np.float32), src, dst, cfg)
    nc = build_kernel(cfg)
    from concourse.bass_utils import run_bass_kernel_spmd
    res = run_bass_kernel_spmd(nc, maps, list(range(cfg.n_cores)))
    outs = [res.results[k]["out"] for k in range(cfg.n_cores)]
    full_slots = np.concatenate(outs, axis=0)          # [N_pad, HF] slot-ordered
    full = full_slots[cfg.slot_of]                     # [N, HF] node-ordered
    return full.reshape(cfg.N, H, F).astype(np.float32)


# revision 3
# speedup vs baseline: 1.1137x; 1.1137x over previous
"""GATv2 + Bessel edge-softmax kernel for TRN2, 8-core SPMD, dst-sharded. v2.

Key structure (vs v1 baseline):
  - Degree-balanced node->slot permutation equalizes edges per 128-node window.
  - Feature tables hold 512B rows: [feat (f,h)-transposed 128 | q=sum_f attn*feat
    (4) | pad].  512B rows cost the same DMA as 256B (sub-512B pays 2x latency),
    so the q columns ride free.
  - PReLU linearized: attn . prelu(c*s) = p*c*(Q) + q*|c|*(sum_f attn*|s|) with
    Q = qs[src]+qd[dst] gathered, c1=p*c / c2=q*|c| host-precomputed per edge.
  - (f,h)-transposed layout lets the per-edge scalars (ex) multiply at DVE 2x
    via mid-dim broadcast views; no ACT materializations of big broadcasts.
  - One-hot scatter matrix built on the Pool engine; abs on Activation engine.
  - 48KB SWDGE scratch ring -> whole-group gather calls (4 calls/window).
"""
import sys
sys.path.insert(0, "/opt/trn_rl_repo")
import heapq
import numpy as np
import ml_dtypes
import concourse.bass as bass
import concourse.tile as tile
from concourse import bacc, mybir
from concourse.bass import ts
from contextlib import ExitStack

F32 = mybir.dt.float32
BF = mybir.dt.bfloat16
I16 = mybir.dt.int16
BFNP = ml_dtypes.bfloat16

CUTOFF = 4.0
P_ENV = 7
H, F, HF, IN = 4, 32, 128, 128
ROW = 256          # table row stride/gather elem (in bf16 elems) = 512B
QC = HF            # column where q values start in a row
NCOL = HF + H      # used columns per row


class Cfg:
    def __init__(self, N, n_cores, NV, B_lo, B_hi, V_lo):
        self.N = N
        self.n_cores = n_cores
        self.NV = NV
        self.NW = NV // 128
        self.N_pad = NV * n_cores
        self.B_lo = B_lo
        self.B_hi = B_hi
        self.B = B_lo + B_hi
        self.V_lo = V_lo
        self.V_hi = self.N_pad - V_lo
        assert self.V_lo % 128 == 0 and self.V_lo <= 32768
        assert self.V_hi <= 32768
        self.scratch = getattr(Cfg, "SCRATCH", 16384)  # power-of-2 ring carveout
        # SWDGE ucode caps one gather call at 1024 descriptors (8 blocks)
        GB = min(8, self.scratch // 16 // 128 // getattr(Cfg, "RING_DIV", 1))
        plan = []                       # (which, blk0, nblk, idx_col0)
        col = 0
        for which, total in (("lo", self.B_lo), ("hi", self.B_hi), ("er", self.B)):
            n = (total + GB - 1) // GB
            base, rem = total // n, total % n
            b0 = 0
            for i in range(n):
                nb = base + (1 if i < rem else 0)
                plan.append((which, b0, nb, col))
                col += 8 * nb
                b0 += nb
        self.plan = plan
        self.S = col


def balance_slots(dst, deg_lo, deg_hi, N, n_cores, NV):
    """Assign nodes to (window, partition) slots, vector-LPT balancing the
    per-window lo-edge and hi-edge counts simultaneously (lazy max-norm heap)."""
    NW = NV // 128
    n_win = n_cores * NW
    deg = deg_lo + deg_hi
    avg_lo = max(1.0, deg_lo.sum() / n_win)
    avg_hi = max(1.0, deg_hi.sum() / n_win)
    order = np.argsort(-deg, kind="stable")
    lo_s = np.zeros(n_win)
    hi_s = np.zeros(n_win)
    counts = np.zeros(n_win, np.int32)
    key = lambda wi: max(lo_s[wi] / avg_lo, hi_s[wi] / avg_hi)
    heap = [(0.0, wi) for wi in range(n_win)]
    heapq.heapify(heap)
    slot_of = np.empty(N, np.int64)
    for n in order:
        while True:
            k, wi = heapq.heappop(heap)
            kk = key(wi)
            if heap and kk > heap[0][0] + 1e-12:
                heapq.heappush(heap, (kk, wi))
                continue
            break
        slot_of[n] = wi * 128 + counts[wi]
        counts[wi] += 1
        lo_s[wi] += deg_lo[n]
        hi_s[wi] += deg_hi[n]
        if counts[wi] < 128:
            heapq.heappush(heap, (key(wi), wi))
    return slot_of


def pick_cfg(src, dst, N, n_cores=8):
    NV = ((N + n_cores * 128 - 1) // (n_cores * 128)) * 128
    N_pad = NV * n_cores
    V_lo = min(32768, N_pad)
    assert N_pad - V_lo <= 32768
    srcv, dstv = np.asarray(src), np.asarray(dst)
    is_lo = srcv < V_lo
    deg_lo = np.bincount(dstv[is_lo], minlength=N).astype(np.float64)
    deg_hi = np.bincount(dstv[~is_lo], minlength=N).astype(np.float64)
    slot_of = balance_slots(dstv, deg_lo, deg_hi, N, n_cores, NV)
    dslot = slot_of[dstv]
    win = dslot // 128
    n_lo = np.bincount(win[is_lo], minlength=n_cores * (NV // 128))
    n_hi = np.bincount(win[~is_lo], minlength=n_cores * (NV // 128))
    B_lo = int((n_lo.max() + 127) // 128)
    B_hi = int(max(1, (n_hi.max() + 127) // 128))
    cfg = Cfg(N, n_cores, NV, B_lo, B_hi, V_lo)
    cfg.slot_of = slot_of
    return cfg


def _coeff(distance, frequencies):
    d = (distance.astype(np.float64) / CUTOFF)[:, None]
    d7 = d ** P_ENV
    A = -(P_ENV + 1) * (P_ENV + 2) / 2.0
    Bc = float(P_ENV * (P_ENV + 2))
    C = -P_ENV * (P_ENV + 1) / 2.0
    env = d + A * d7 + Bc * (d7 * d) + C * (d7 * d * d)
    return (env * np.sin(frequencies.astype(np.float64) * d)).astype(np.float64)


def wrap_idx(vals, nslots):
    """SWDGE idx layout for ONE gather call of `nslots` idxs:
    [16, nslots/16] wrap replicated over the 8 gpsimd groups -> [128, nslots/16]."""
    a = np.zeros(nslots, np.int32)
    a[: len(vals)] = vals
    w = a.reshape(nslots // 16, 16).T.astype(np.int16)
    return np.tile(w, (8, 1))


def _perm_cols(W, b, attn):
    """[IN,NCOL] projection matrix: cols f*H+h = W[h*F+f,:]; cols QC+h = attn-dot."""
    Wext = np.zeros((IN, ROW), np.float64)
    bext = np.zeros((ROW,), np.float64)
    W = W.astype(np.float64)
    b = b.astype(np.float64)
    at = attn.reshape(H, F).astype(np.float64)
    for h in range(H):
        for f in range(F):
            Wext[:, f * H + h] = W[h * F + f, :]
            bext[f * H + h] = b[h * F + f]
        Wext[:, QC + h] = (at[h, :, None] * W[h * F : (h + 1) * F, :]).sum(axis=0)
        bext[QC + h] = (at[h] * b[h * F : (h + 1) * F]).sum()
    return Wext, bext


def host_prep(x, distance, W_src, b_src, W_dst, b_dst, attn, prelu_alpha,
              frequencies, src, dst, cfg: Cfg):
    c = cfg
    src = np.asarray(src).astype(np.int64)
    dst = np.asarray(dst).astype(np.int64)
    slot_of = c.slot_of
    dslot = slot_of[dst]
    order = np.argsort(dslot, kind="stable")
    src_s, dslot_s, dis_s = src[order], dslot[order], np.asarray(distance)[order]

    cfg.has_bias = bool(np.any(np.asarray(b_src)) or np.any(np.asarray(b_dst)))
    coeff = _coeff(dis_s, np.asarray(frequencies))          # [E, H] float64
    alpha = np.asarray(prelu_alpha).astype(np.float64)
    p = (1.0 + alpha) / 2.0
    q = (1.0 - alpha) / 2.0
    c1_all = (p[None, :] * coeff).astype(np.float32)
    c2_all = (q[None, :] * np.abs(coeff)).astype(np.float32)

    # node -> slot inverse for x column placement
    node_of = np.full(c.N_pad, -1, np.int64)
    node_of[slot_of] = np.arange(c.N)

    xT = np.zeros((IN, c.N_pad), BFNP)
    xT[:, : c.N] = np.asarray(x).T.astype(BFNP)

    Wse, bse = _perm_cols(np.asarray(W_src), np.asarray(b_src), np.asarray(attn))
    Wde, bde = _perm_cols(np.asarray(W_dst), np.asarray(b_dst), np.asarray(attn))
    attn_t = np.zeros((1, HF), np.float32)
    at = np.asarray(attn).reshape(H, F)
    for h in range(H):
        for f in range(F):
            attn_t[0, f * H + h] = at[h, f]

    smalls = dict(
        w_src_e=Wse[:, :NCOL].astype(BFNP),
        w_dst_e=Wde[:, :NCOL].astype(BFNP),
        b_src_e=bse[None, :NCOL].astype(BFNP),
        b_dst_e=bde[None, :NCOL].astype(BFNP),
        attn_t_row=attn_t.astype(BFNP),
    )

    core_of = dslot_s // c.NV
    maps = []
    for k in range(c.n_cores):
        sel = core_of == k
        sk = src_s[sel]
        lk = dslot_s[sel] - k * c.NV          # local slot
        c1k, c2k = c1_all[sel], c2_all[sel]
        win = lk // 128
        part = lk % 128

        idx = np.zeros((c.NW, 128, c.S), np.int16)
        dstw = np.full((128, c.NW, c.B), -1.0, BFNP)
        c12 = np.zeros((128, c.NW, c.B, 2 * H), np.float32)

        for w in range(c.NW):
            wsel = win == w
            ws, wp = sk[wsel], part[wsel]
            w1, w2 = c1k[wsel], c2k[wsel]
            lo = ws < c.V_lo
            n_lo, n_hi = int(lo.sum()), int((~lo).sum())
            assert n_lo <= c.B_lo * 128 and n_hi <= c.B_hi * 128

            # edge slot j -> (p=j%128, blk=j//128); lo slots then hi slots
            ers = np.zeros(c.B * 128, np.int64)   # er gather idx (local dst slot)
            jl = np.arange(n_lo)
            dstw[jl % 128, w, jl // 128] = wp[lo]
            c12[jl % 128, w, jl // 128, 0:H] = w1[lo]
            c12[jl % 128, w, jl // 128, H:] = w2[lo]
            ers[jl] = w * 128 + wp[lo]
            jh = np.arange(n_hi)
            dstw[jh % 128, w, c.B_lo + jh // 128] = wp[~lo]
            c12[jh % 128, w, c.B_lo + jh // 128, 0:H] = w1[~lo]
            c12[jh % 128, w, c.B_lo + jh // 128, H:] = w2[~lo]
            ers[c.B_lo * 128 + jh] = w * 128 + wp[~lo]

            els_lo = np.zeros(c.B_lo * 128, np.int64)
            els_lo[:n_lo] = ws[lo]
            els_hi = np.zeros(c.B_hi * 128, np.int64)
            els_hi[:n_hi] = ws[~lo] - c.V_lo
            pools = {"lo": els_lo, "hi": els_hi, "er": ers}
            for which, b0, nb, col in c.plan:
                vals = pools[which][b0 * 128 : (b0 + nb) * 128]
                idx[w, :, col : col + 8 * nb] = wrap_idx(vals, nb * 128)

        m = dict(smalls)
        xT_own = np.zeros((IN, c.NV), BFNP)
        own_nodes = node_of[k * c.NV : (k + 1) * c.NV]
        valid = own_nodes >= 0
        xT_own[:, valid] = xT[:, own_nodes[valid]]
        m.update(xT=xT, xT_own=xT_own, idx=idx, dstw=dstw, c12=c12)
        maps.append(m)
    return maps


def build_kernel(c: Cfg):
    nc = bacc.Bacc("TRN2", target_bir_lowering=False, debug=False,
                   dynamic_dma_scratch_size=c.scratch)
    dp = nc.declare_dram_parameter
    xT = dp("xT", [IN, c.N_pad], BF, isOutput=False)
    xT_own = dp("xT_own", [IN, c.NV], BF, isOutput=False)
    w_src_e = dp("w_src_e", [IN, NCOL], BF, isOutput=False)
    w_dst_e = dp("w_dst_e", [IN, NCOL], BF, isOutput=False)
    b_src_e = dp("b_src_e", [1, NCOL], BF, isOutput=False)
    b_dst_e = dp("b_dst_e", [1, NCOL], BF, isOutput=False)
    attn_t_row = dp("attn_t_row", [1, HF], BF, isOutput=False)
    idx_d = dp("idx", [c.NW, 128, c.S], I16, isOutput=False)
    dstw = dp("dstw", [128, c.NW, c.B], BF, isOutput=False)
    c12d = dp("c12", [128, c.NW, c.B, 2 * H], F32, isOutput=False)
    out = dp("out", [c.NV, HF], F32, isOutput=True)

    if getattr(c, "host_tables", False):
        feat_lo = dp("feat_lo", [c.V_lo, ROW], BF, isOutput=False)
        feat_hi = dp("feat_hi", [c.V_hi, ROW], BF, isOutput=False)
        feat_dst = dp("feat_dst", [c.NV, ROW], BF, isOutput=False)
        c.skip_proj = True
    else:
        feat_lo = nc.dram_tensor("feat_lo", [c.V_lo, ROW], BF)
        feat_hi = nc.dram_tensor("feat_hi", [c.V_hi, ROW], BF)
        feat_dst = nc.dram_tensor("feat_dst", [c.NV, ROW], BF)

    debug = getattr(c, "debug", False)
    if debug:
        dbg_el2 = dp("dbg_el2", [128, c.B, ROW], BF, isOutput=True)
        dbg = {name: dp(f"dbg_{name}", shape, dt, isOutput=True)
               for name, shape, dt in [
                   ("el", [128, c.B, ROW], BF), ("er", [128, c.B, ROW], BF),
                   ("s", [128, c.B, HF], BF), ("sh2", [128, c.B, 32], BF),
                   ("bred", [128, c.B, H], F32), ("qa", [128, c.B, H], BF),
                   ("score", [128, c.B, H], F32), ("msgex", [128, c.B, NCOL], BF),
                   ("oh", [128, c.B, 128], BF), ("U", [128, NCOL], F32)]}

    mm = mybir.AluOpType
    AF = mybir.ActivationFunctionType

    def apv(base_ap, dims):
        """Rebuild an AP view on the same tensor/offset with custom free dims."""
        return bass.AP(tensor=base_ap.tensor, offset=base_ap.offset,
                       ap=[list(base_ap.ap[0])] + [list(d) for d in dims])

    with tile.TileContext(nc, pool_alloc_mode="queue") as tc, ExitStack() as ctx:
        con = ctx.enter_context(tc.tile_pool(name="con", bufs=1))
        attn_rep = con.tile([128, HF], BF)
        nc.sync.dma_start(
            out=attn_rep[:],
            in_=bass.AP(tensor=attn_t_row.ap().tensor, offset=attn_t_row.ap().offset,
                        ap=[[0, 128], [1, HF]]))
        iota_i = con.tile([128, 128], mybir.dt.int32)
        nc.gpsimd.iota(iota_i[:], pattern=[[1, 128]], base=0, channel_multiplier=0)
        iota_f = con.tile([128, 128], BF)
        nc.vector.tensor_copy(out=iota_f[:], in_=iota_i[:])
        ones_sb = con.tile([1, 128], BF)
        nc.vector.memset(ones_sb[:], 1.0)
        dstw_sb = con.tile([128, c.NW, c.B], BF)
        nc.sync.dma_start(out=dstw_sb[:], in_=dstw[:])

        # --- projections: feat tables with q columns ---
        skip_proj = getattr(c, "skip_proj", False)
        skip_edges = getattr(c, "skip_edges", False)
        with tc.tile_pool(name="proj", bufs=3) as pp, \
             tc.tile_pool(name="projp", bufs=2, space="PSUM") as ppp:
            w_src_sb = pp.tile([IN, NCOL], BF, tag="wsrc")
            nc.sync.dma_start(out=w_src_sb[:], in_=w_src_e[:])
            w_dst_sb = pp.tile([IN, NCOL], BF, tag="wdst")
            nc.sync.dma_start(out=w_dst_sb[:], in_=w_dst_e[:])
            b_src_sb = pp.tile([1, NCOL], BF, tag="bsrc")
            nc.sync.dma_start(out=b_src_sb[:], in_=b_src_e[:])
            b_dst_sb = pp.tile([1, NCOL], BF, tag="bdst")
            nc.sync.dma_start(out=b_dst_sb[:], in_=b_dst_e[:])

            last_write = {}       # table name -> last write DMA instruction

            def project(xt_ap, n_tiles, w_sb, b_sb, dests, has_bias):
                G = 4
                for g0 in range(0, n_tiles, G):
                    g = min(G, n_tiles - g0)
                    xt_t = pp.tile([128, G * 128], BF, tag="xt")
                    nc.sync.dma_start(out=xt_t[:, : g * 128],
                                      in_=xt_ap[:, g0 * 128 : (g0 + g) * 128])
                    # 256-col (1KB) stride per tile keeps every matmul output
                    # region 512B-aligned in PSUM
                    ps = ppp.tile([128, G, 256], F32)
                    for t in range(g):
                        nc.tensor.matmul(ps[:, t, :NCOL], lhsT=xt_t[:, ts(t, 128)],
                                         rhs=w_sb[:], start=True, stop=not has_bias)
                        if has_bias:
                            nc.tensor.matmul(ps[:, t, :NCOL], lhsT=ones_sb[:],
                                             rhs=b_sb[:], start=False, stop=True)
                    # full-ROW rows (pad cols carry garbage, never read) so the
                    # table write is a contiguous row-major DMA
                    ft = pp.tile([128, G, ROW], BF, tag="ft")
                    nc.scalar.copy(
                        out=ft[:, :g, :NCOL],
                        in_=ps[:, :g, :NCOL])
                    for dram, t0, nt in dests:
                        a = max(g0, t0)
                        b = min(g0 + g, t0 + nt)
                        if a < b:
                            rows = dram[(a - t0) * 128 : (b - t0) * 128, :]
                            last_write[dram.name] = nc.sync.dma_start(
                                out=rows.rearrange("(t p) f -> p t f", p=128),
                                in_=ft[:, a - g0 : b - g0, :])
            if not skip_proj:
                hb = getattr(c, "has_bias", True)
                project(xT_own.ap(), c.NV // 128, w_dst_sb, b_dst_sb,
                        [(feat_dst, 0, c.NV // 128)], hb)
                project(xT.ap(), c.N_pad // 128, w_src_sb, b_src_sb,
                        [(feat_lo, 0, c.V_lo // 128),
                         (feat_hi, c.V_lo // 128, c.V_hi // 128)], hb)


        # --- edge phase ---
        ep = ctx.enter_context(tc.tile_pool(name="edge", bufs=3))
        wp = ctx.enter_context(tc.tile_pool(name="work", bufs=1))
        mp = ctx.enter_context(tc.tile_pool(name="mpool", bufs=2))
        dp2 = ctx.enter_context(tc.tile_pool(name="dwpool", bufs=2))
        op_ = ctx.enter_context(tc.tile_pool(name="outp", bufs=2))
        up = ctx.enter_context(tc.tile_pool(name="upsum", bufs=2, space="PSUM"))

        B = c.B
        for w in range(c.NW if not skip_edges else 0):
            id_t = ep.tile([128, c.S], I16, tag="idx")
            nc.sync.dma_start(out=id_t[:], in_=idx_d[w])
            c12w = ep.tile([128, B, 2 * H], F32, tag="c12w")
            nc.sync.dma_start(out=c12w[:], in_=c12d[:, w])

            el = ep.tile([128, B, ROW], BF, tag="el")
            er = ep.tile([128, B, ROW], BF, tag="er")
            for which, b0, nb, col in c.plan:
                if which == "lo":
                    dst_sl, tab = el[:, b0 : b0 + nb, :], feat_lo
                elif which == "hi":
                    dst_sl, tab = el[:, c.B_lo + b0 : c.B_lo + b0 + nb, :], feat_hi
                else:
                    dst_sl, tab = er[:, b0 : b0 + nb, :], feat_dst
                g_inst = nc.gpsimd.dma_gather(dst_sl, tab[:],
                                              id_t[:, col : col + 8 * nb],
                                              nb * 128, nb * 128, ROW)
                # dma_gather's DRAM source is not dependency-tracked by the
                # tile framework: order it after that table's final write
                lw = last_write.get(tab.name)
                if lw is not None:
                    tile.add_dep_helper(
                        g_inst.ins if hasattr(g_inst, "ins") else g_inst,
                        lw.ins if hasattr(lw, "ins") else lw,
                        reason="gather after table write")

            # one-hot: oh[p, b, n] = (iota[n] == dstw[p, b]); dstw broadcast
            # materialized on ACT so the DVE is_equal stays in 2x mode
            dwbig = dp2.tile([128, B, 128], BF, tag="dwbig")
            dw = dstw_sb[:, w, :]                       # [128, B]
            nc.scalar.copy(out=dwbig[:],
                           in_=apv(dw, [list(dw.ap[1]), [0, 128]]))
            oh = mp.tile([128, B, 128], BF, tag="oh")
            nc.vector.tensor_tensor(
                out=oh[:],
                in0=apv(iota_f[:], [[0, B], [1, 128]]),
                in1=dwbig[:],
                op=mm.is_equal)

            # s = el + er (feat part), DVE 2x; then |s| in place on ACT;
            # then *= attn in place (2x, mid-bcast of replicated attn row)
            s_t = wp.tile([128, B, HF], BF, tag="s")
            nc.vector.tensor_add(s_t[:], el[:, :, :HF], er[:, :, :HF])
            nc.scalar.activation(s_t[:], s_t[:], AF.Abs)
            nc.vector.tensor_tensor(
                out=s_t[:], in0=s_t[:],
                in1=apv(attn_rep[:], [[0, B], [1, HF]]), op=mm.mult)
            u_t = s_t
            # pairwise halving (f-major layout: f and f+16 are 64 cols apart)
            sh1 = wp.tile([128, B, 64], BF, tag="sh1")
            nc.vector.tensor_add(sh1[:], u_t[:, :, :64], u_t[:, :, 64:])
            sh2 = wp.tile([128, B, 32], BF, tag="sh2")
            nc.vector.tensor_add(sh2[:], sh1[:, :, :32], sh1[:, :, 32:])
            # reduce over f (strided view [p, B, h, f8]) -> Bred [128, B, H] f32
            bred = wp.tile([128, B, H], F32, tag="bred")
            sh2b = sh2[:]
            nc.vector.tensor_reduce(
                out=bred[:],
                in_=bass.AP(tensor=sh2b.tensor, offset=sh2b.offset,
                            ap=[list(sh2b.ap[0]), [32, B], [1, H], [H, 8]]),
                axis=mybir.AxisListType.X, op=mm.add)

            # score = c1*(qs+qd) + c2*Bred   [128, B, H] f32
            qa = wp.tile([128, B, H], BF, tag="qa")
            nc.vector.tensor_add(qa[:], el[:, :, QC:NCOL], er[:, :, QC:NCOL])
            sc1 = wp.tile([128, B, H], F32, tag="sc1")
            nc.vector.tensor_tensor(out=sc1[:], in0=qa[:], in1=c12w[:, :, 0:H],
                                    op=mm.mult)
            score = wp.tile([128, B, H], F32, tag="score")
            # fold the final add into the c2 multiply via scalar_tensor_tensor:
            # score = (bred * 1.0 ... ) no scalar slot for c2 — keep 2-op form
            sc2 = wp.tile([128, B, H], F32, tag="sc2")
            nc.vector.tensor_tensor(out=sc2[:], in0=bred[:], in1=c12w[:, :, H:],
                                    op=mm.mult)
            nc.vector.tensor_add(score[:], sc1[:], sc2[:])

            # msgex: cols 0:HF = el*ex, cols HF:NCOL = ex
            msgex = mp.tile([128, B, NCOL], BF, tag="msgex")
            nc.scalar.activation(msgex[:, :, QC:NCOL], score[:], AF.Exp)
            exv = msgex[:, :, QC:NCOL]
            nc.vector.tensor_tensor(
                out=msgex[:, :, :HF], in0=el[:, :, :HF],
                in1=bass.AP(tensor=exv.tensor, offset=exv.offset,
                            ap=[list(exv.ap[0]), [NCOL, B], [0, F], [1, H]]),
                op=mm.mult)

            # scatter via one-hot matmuls
            U = up.tile([128, NCOL], F32, tag="U")
            for b in range(B):
                nc.tensor.matmul(U[:], lhsT=oh[:, b, :], rhs=msgex[:, b, :],
                                 start=(b == 0), stop=(b == B - 1))

            if debug and w == 0:
                for nm, t in [("el", el), ("er", er), ("s", u_t), ("sh2", sh2),
                              ("bred", bred), ("qa", qa), ("score", score),
                              ("msgex", msgex), ("oh", oh)]:
                    nc.sync.dma_start(out=dbg[nm][:], in_=t[:])
                uc = op_.tile([128, NCOL], F32, tag="uc")
                nc.scalar.copy(out=uc[:], in_=U[:])
                nc.sync.dma_start(out=dbg["U"][:], in_=uc[:])

            inv = op_.tile([128, H], F32, tag="inv")
            nc.vector.tensor_scalar_max(inv[:], U[:, QC:NCOL], 1e-30)
            nc.vector.reciprocal(inv[:], inv[:])
            # ot holds (h,f) layout: strided DVE out unscrambles the (f,h) cols
            ot = op_.tile([128, HF], F32, tag="ot")
            otb = ot[:]
            nc.vector.tensor_tensor(
                out=bass.AP(tensor=otb.tensor, offset=otb.offset,
                            ap=[list(otb.ap[0]), [1, F], [F, H]]),
                in0=U[:, :HF],
                in1=apv(inv[:], [[0, F], [1, H]]), op=mm.mult)
            nc.sync.dma_start(out=out[ts(w, 128)], in_=ot[:])

        if debug:
            # re-gather window 0's el at the very end: distinguishes a
            # write/gather race from permanently corrupt table rows
            id2 = ep.tile([128, c.S], I16, tag="idx")
            nc.sync.dma_start(out=id2[:], in_=idx_d[0])
            el2 = ep.tile([128, B, ROW], BF, tag="el")
            for which, b0, nb, col in c.plan:
                if which == "lo":
                    nc.gpsimd.dma_gather(el2[:, b0 : b0 + nb, :], feat_lo[:],
                                         id2[:, col : col + 8 * nb],
                                         nb * 128, nb * 128, ROW)
                elif which == "hi":
                    nc.gpsimd.dma_gather(el2[:, c.B_lo + b0 : c.B_lo + b0 + nb, :],
                                         feat_hi[:], id2[:, col : col + 8 * nb],
                                         nb * 128, nb * 128, ROW)
            nc.sync.dma_start(out=dbg_el2[:], in_=el2[:])

    nc.compile()
    return nc


def kernel(**inputs) -> np.ndarray:
    x = np.asarray(inputs["x"], np.float32)
    src = np.asarray(inputs["src"]).astype(np.int64)
    dst = np.asarray(inputs["dst"]).astype(np.int64)
    cfg = pick_cfg(src, dst, x.shape[0], 8)
    maps = host_prep(
        x, np.asarray(inputs["distance"], np.float32),
        np.asarray(inputs["W_src"], np.float32), np.asarray(inputs["b_src"], np.float32),
        np.asarray(inputs["W_dst"], np.float32), np.asarray(inputs["b_dst"], np.float32),
        np.asarray(inputs["attn"], np.float32), np.asarray(inputs["prelu_alpha"], np.float32),
        np.asarray(inputs["frequencies"], np.float32), src, dst, cfg)
    nc = build_kernel(cfg)
    from concourse.bass_utils import run_bass_kernel_spmd
    res = run_bass_kernel_spmd(nc, maps, list(range(cfg.n_cores)))
    outs = [res.results[k]["out"] for k in range(cfg.n_cores)]
    full_slots = np.concatenate(outs, axis=0)          # [N_pad, HF] slot-ordered
    full = full_slots[cfg.slot_of]                     # [N, HF] node-ordered
    return full.reshape(cfg.N, H, F).astype(np.float32)
